# revision 1
# baseline (speedup 1.0000x reference)
"""Trainium2 Bass kernel: windowed-LSTM local attention + linear head (LBNER).

Strategy
--------
Data-parallel over batch: B=8 sequences -> 8 NeuronCores, one sequence each.
Per core everything is laid out feature-on-partitions, L=512 on the free dim:

  xT            [768, 512]      (6 SBUF tiles of [128, 512])
  gates/P       [3072, 512]     (24 tiles of [128, 512])
  h, c          [768, 512]      (6 tiles each)

For each window size w in (3,5,7):
  P = Wih @ xT + (b_ih + b_hh)  computed ONCE (shared by all w steps; step t
  just reads P shifted by (t - w//2) columns).  Step t updates only the column
  range [s, e) that is "valid" for that offset, so out-of-range window slots
  never touch state -- this reproduces the reference's mask semantics with no
  mask tensors at all.  Step 0 has h=0 so its hidden matmul is skipped.

Recurrence per step (t >= 1):  gates_psum = WhhT.T @ h  (24 [128,512] psum
tiles, 6 K-chunks each, bf16 x bf16 -> fp32 PSUM), then per d-chunk:
  pre_g = psum + P_shift (DVE)  ->  sigmoid/tanh (ACT)  ->
  c = f*c + i*g (DVE, fp32)     ->  h = o * tanh(c) (DVE, bf16)

After the 3 windows: attn logits via elementwise mul + ones-matmul column
reduction, 3-way softmax on [1,512] rows, attention weights broadcast across
partitions with a K=1 outer-product matmul, and the residual is folded into
the head matmul: logits = lin_w @ xT + lin_w @ (sum_k attn_k * locals_k) + b.

Weights are converted to bf16 on the host; matmul accumulation is fp32 in
PSUM; the cell state c stays fp32; attention/head matmuls run plain fp32.
"""

import math
import numpy as np
import ml_dtypes

import concourse.bacc as bacc
import concourse.bass as bass
import concourse.tile as tile
from concourse import mybir
from concourse import bass_utils

B, L, D = 8, 512, 768
NL = 9
WINDOWS = (3, 5, 7)
NW = len(WINDOWS)
G4 = 4 * D          # 3072
P = 128
ND = D // P         # 6 d-chunks
NM = G4 // P        # 24 gate-chunks
N_CORES = 8

F32 = mybir.dt.float32
F32R = mybir.dt.float32r
BF16 = mybir.dt.bfloat16
AF = mybir.ActivationFunctionType


def _emit(tc, io):
    nc = tc.nc
    from contextlib import ExitStack

    with ExitStack() as ctx:
        const = ctx.enter_context(tc.tile_pool(name="const", bufs=1))
        wpool = ctx.enter_context(tc.tile_pool(name="wpool", bufs=1))
        ppool = ctx.enter_context(tc.tile_pool(name="ppool", bufs=1))
        state = ctx.enter_context(tc.tile_pool(name="state", bufs=1))
        post = ctx.enter_context(tc.tile_pool(name="post", bufs=8))
        tmp = ctx.enter_context(tc.tile_pool(name="tmp", bufs=6))
        attn = ctx.enter_context(tc.tile_pool(name="attn", bufs=7))
        logp = ctx.enter_context(tc.tile_pool(name="logp", bufs=1))
        psum = ctx.enter_context(tc.tile_pool(name="psum", bufs=8, space="PSUM"))

        # ---- constants / inputs resident in SBUF ----
        xf = []   # x.T fp32, for attention dot + residual head matmul
        xb = []   # x.T bf16, rhs of the input projections
        for dc in range(ND):
            t_f = const.tile([P, L], F32, tag=f"xf{dc}")
            nc.sync.dma_start(t_f, io["xf"].ap()[dc * P:(dc + 1) * P, :])
            xf.append(t_f)
            t_b = const.tile([P, L], BF16, tag=f"xb{dc}")
            nc.sync.dma_start(t_b, io["xb"].ap()[dc * P:(dc + 1) * P, :])
            xb.append(t_b)

        # combined LSTM bias, laid out [128, NW, NM]: partition p, window k,
        # gate-chunk m  <-  bias[k, m*128 + p]
        bias_sb = const.tile([P, NW, NM], F32, tag="bias")
        nc.sync.dma_start(
            bias_sb, io["bias"].ap().rearrange("k (m p) -> p k m", p=P)
        )

        lw = []
        for dc in range(ND):
            t = const.tile([P, NL], F32, tag=f"lw{dc}")
            nc.sync.dma_start(t, io["lwt"].ap()[dc * P:(dc + 1) * P, :])
            lw.append(t)
        lb_sb = const.tile([NL, 1], F32, tag="lb")
        nc.sync.dma_start(lb_sb, io["lb"].ap().rearrange("(c o) -> c o", o=1))

        ident_sb = const.tile([P, P], BF16, tag="ident")
        nc.sync.dma_start(ident_sb, io["ident"].ap())

        ones_col = const.tile([P, 1], F32, tag="ones_col")
        nc.vector.memset(ones_col, 1.0)
        ones_row = const.tile([1, P], F32, tag="ones_row")
        nc.vector.memset(ones_row, 1.0)

        locals_k = []   # per window: list of 6 bf16 [128, 512] tiles (final h)
        a_sb = []       # per-window attention logit rows [1, 512]
        inv_sqrt_d = 1.0 / math.sqrt(D)

        for k, w in enumerate(WINDOWS):
            hw_ = w // 2

            # ---- weights for this window (2 rotating 9.4MB slots) ----
            wih = []
            for kc in range(ND):
                t = wpool.tile([P, G4], BF16, tag=f"A{kc}")
                nc.sync.dma_start(t, io["wih"].ap()[k, kc * P:(kc + 1) * P, :])
                wih.append(t)
            whh = []
            for kc in range(ND):
                t = wpool.tile([P, G4], BF16, tag=f"B{kc}")
                nc.sync.dma_start(t, io["whh"].ap()[k, kc * P:(kc + 1) * P, :])
                whh.append(t)

            # ---- input projection: P_m = bias_m + sum_kc Wih[kc,m].T @ xT ----
            Pt = []
            for m in range(NM):
                ps = psum.tile([P, L], F32, tag="g")
                for kc in range(ND):
                    nc.tensor.matmul(
                        ps,
                        lhsT=wih[kc][:, m * P:(m + 1) * P],
                        rhs=xb[kc][:],
                        start=(kc == 0),
                        stop=(kc == ND - 1),
                    )
                pt = ppool.tile([P, L], BF16, tag=f"P{m}")
                nc.vector.tensor_scalar_add(pt, ps, bias_sb[:, k, m:m + 1])
                Pt.append(pt)

            # ---- state init ----
            c = []
            h = []
            for dc in range(ND):
                ct = state.tile([P, L], F32, tag=f"c{dc}")
                nc.gpsimd.memset(ct, 0.0)
                c.append(ct)
                ht = state.tile([P, L], BF16, tag=f"loc{k}_{dc}")
                nc.gpsimd.memset(ht, 0.0)
                h.append(ht)

            # ---- recurrence over window positions ----
            for t in range(w):
                off = t - hw_
                s = max(0, -off)
                e = min(L, L - off)
                n = e - s

                if t == 0:
                    # h == 0: gates come straight from P (bias included)
                    for dc in range(ND):
                        i_t = post.tile([P, L], BF16, tag="post")
                        nc.scalar.activation(
                            i_t[:, :n], Pt[0 + dc][:, s + off:e + off], AF.Sigmoid
                        )
                        g_t = post.tile([P, L], BF16, tag="post")
                        nc.scalar.activation(
                            g_t[:, :n], Pt[12 + dc][:, s + off:e + off], AF.Tanh
                        )
                        o_t = post.tile([P, L], BF16, tag="post")
                        nc.scalar.activation(
                            o_t[:, :n], Pt[18 + dc][:, s + off:e + off], AF.Sigmoid
                        )
                        nc.vector.tensor_mul(c[dc][:, s:e], i_t[:, :n], g_t[:, :n])
                        tc_t = post.tile([P, L], BF16, tag="post")
                        nc.scalar.activation(tc_t[:, :n], c[dc][:, s:e], AF.Tanh)
                        nc.vector.tensor_mul(h[dc][:, s:e], o_t[:, :n], tc_t[:, :n])
                    continue

                for dc in range(ND):
                    # 4 gate psum tiles for this d-chunk: i, f, g, o.
                    # P_shift (incl. bias) is folded into the accumulation
                    # with an identity matmul, so ACT reads gates from PSUM.
                    gp = []
                    for base in (0, 6, 12, 18):
                        m = base + dc
                        ps = psum.tile([P, L], F32, tag="g")
                        nc.tensor.matmul(
                            ps[:, s:e],
                            lhsT=ident_sb[:],
                            rhs=Pt[m][:, s + off:e + off],
                            start=True,
                            stop=False,
                        )
                        for kc in range(ND):
                            nc.tensor.matmul(
                                ps[:, s:e],
                                lhsT=whh[kc][:, m * P:(m + 1) * P],
                                rhs=h[kc][:, s:e],
                                start=False,
                                stop=(kc == ND - 1),
                            )
                        gp.append(ps)

                    acts = []
                    for gi, fn in enumerate(
                        (AF.Sigmoid, AF.Sigmoid, AF.Tanh, AF.Sigmoid)
                    ):
                        a = post.tile([P, L], BF16, tag="post")
                        nc.scalar.activation(a[:, :n], gp[gi][:, s:e], fn)
                        acts.append(a)
                    i_t, f_t, g_t, o_t = acts

                    t1 = tmp.tile([P, L], F32, tag="tmp")
                    nc.vector.tensor_mul(t1[:, :n], i_t[:, :n], g_t[:, :n])
                    t2 = tmp.tile([P, L], F32, tag="tmp")
                    nc.vector.tensor_mul(t2[:, :n], f_t[:, :n], c[dc][:, s:e])
                    nc.vector.tensor_add(c[dc][:, s:e], t1[:, :n], t2[:, :n])
                    tc_t = post.tile([P, L], BF16, tag="post")
                    nc.scalar.activation(tc_t[:, :n], c[dc][:, s:e], AF.Tanh)
                    nc.vector.tensor_mul(h[dc][:, s:e], o_t[:, :n], tc_t[:, :n])

            locals_k.append(h)

            # attention dot for this window, overlapped with the next window
            psd = psum.tile([1, L], F32, tag="g")
            for dc in range(ND):
                td = tmp.tile([P, L], F32, tag="tmp")
                nc.vector.tensor_mul(td, xf[dc][:], h[dc][:])
                nc.tensor.matmul(
                    psd,
                    lhsT=ones_col[:],
                    rhs=td[:],
                    start=(dc == 0),
                    stop=(dc == ND - 1),
                )
            ak = attn.tile([1, L], F32, tag=f"ak{k}", bufs=1)
            nc.scalar.activation(ak, psd, AF.Copy, scale=inv_sqrt_d)
            a_sb.append(ak)

        # ---- attention over the 3 window outputs ----
        mx1 = attn.tile([1, L], F32, tag="sm")
        nc.vector.tensor_max(mx1, a_sb[0][:], a_sb[1][:])
        mx2 = attn.tile([1, L], F32, tag="sm")
        nc.vector.tensor_max(mx2, mx1[:], a_sb[2][:])
        d_sb = []
        for k in range(NW):
            d_k = attn.tile([1, L], F32, tag="sm")
            nc.vector.tensor_sub(d_k, a_sb[k][:], mx2[:])
            d_sb.append(d_k)
        e_sb = []
        for k in range(NW):
            ek = attn.tile([1, L], F32, tag="sm")
            nc.scalar.activation(ek, d_sb[k][:], AF.Exp)
            e_sb.append(ek)
        s1 = attn.tile([1, L], F32, tag="sm")
        nc.vector.tensor_add(s1, e_sb[0][:], e_sb[1][:])
        s2 = attn.tile([1, L], F32, tag="sm")
        nc.vector.tensor_add(s2, s1[:], e_sb[2][:])
        r = attn.tile([1, L], F32, tag="sm")
        nc.vector.reciprocal(r, s2[:])

        wb = []   # attention weights broadcast to [128, 512] (PSUM)
        for k in range(NW):
            wk = attn.tile([1, L], F32, tag="sm")
            nc.vector.tensor_mul(wk, e_sb[k][:], r[:])
            pb = psum.tile([P, L], F32, tag="g")
            nc.tensor.matmul(
                pb,
                lhsT=ones_row[:],
                rhs=wk[:],
                start=True,
                stop=True,
            )
            wb.append(pb)

        # ---- head: logits = lin_w @ (x + sum_k attn_k * locals_k) + b ----
        ps_log = psum.tile([NL, L], F32, tag="g")
        for dc in range(ND):
            nc.tensor.matmul(
                ps_log,
                lhsT=lw[dc][:],
                rhs=xf[dc][:],
                start=(dc == 0),
                stop=False,
            )
        for dc in range(ND):
            lf = tmp.tile([P, L], F32, tag="tmp")
            nc.vector.tensor_mul(lf, wb[0][:], locals_k[0][dc][:])
            t3 = tmp.tile([P, L], F32, tag="tmp")
            nc.vector.tensor_mul(t3, wb[1][:], locals_k[1][dc][:])
            lf2 = tmp.tile([P, L], F32, tag="tmp")
            nc.vector.tensor_add(lf2, lf[:], t3[:])
            t4 = tmp.tile([P, L], F32, tag="tmp")
            nc.vector.tensor_mul(t4, wb[2][:], locals_k[2][dc][:])
            lf3 = tmp.tile([P, L], F32, tag="tmp")
            nc.vector.tensor_add(lf3, lf2[:], t4[:])
            nc.tensor.matmul(
                ps_log,
                lhsT=lw[dc][:],
                rhs=lf3[:],
                start=False,
                stop=(dc == ND - 1),
            )
        logits = logp.tile([NL, L], F32, tag="logits")
        nc.scalar.activation(logits, ps_log, AF.Identity, bias=lb_sb[:, 0:1])
        # store transposed: out[l, c] = logits[c, l]
        nc.sync.dma_start(io["out"].ap().rearrange("l c -> c l"), logits[:])


_NC_CACHE = {}


def _get_nc():
    if "nc" not in _NC_CACHE:
        nc = bacc.Bacc("TRN2", target_bir_lowering=False, debug=False)
        io = {
            "xf": nc.dram_tensor("xf", [D, L], F32, kind="ExternalInput"),
            "xb": nc.dram_tensor("xb", [D, L], BF16, kind="ExternalInput"),
            "wih": nc.dram_tensor("wih", [NW, D, G4], BF16, kind="ExternalInput"),
            "whh": nc.dram_tensor("whh", [NW, D, G4], BF16, kind="ExternalInput"),
            "bias": nc.dram_tensor("bias", [NW, G4], F32, kind="ExternalInput"),
            "lwt": nc.dram_tensor("lwt", [D, NL], F32, kind="ExternalInput"),
            "lb": nc.dram_tensor("lb", [NL], F32, kind="ExternalInput"),
            "ident": nc.dram_tensor("ident", [P, P], BF16, kind="ExternalInput"),
            "out": nc.dram_tensor("out", [L, NL], F32, kind="ExternalOutput"),
        }
        with tile.TileContext(nc) as tc:
            _emit(tc, io)
        nc.compile()
        _NC_CACHE["nc"] = nc
    return _NC_CACHE["nc"]


def _in_maps(sequence_output, W_ih, W_hh, b_ih, b_hh, lin_w, lin_b):
    x = np.asarray(sequence_output, np.float32)
    WihT = np.ascontiguousarray(
        np.transpose(np.asarray(W_ih, np.float32), (0, 2, 1))
    ).astype(ml_dtypes.bfloat16)
    WhhT = np.ascontiguousarray(
        np.transpose(np.asarray(W_hh, np.float32), (0, 2, 1))
    ).astype(ml_dtypes.bfloat16)
    biasc = np.asarray(b_ih, np.float32) + np.asarray(b_hh, np.float32)
    lwt = np.ascontiguousarray(np.asarray(lin_w, np.float32).T)
    lb = np.asarray(lin_b, np.float32)
    maps = []
    for b in range(B):
        xT = np.ascontiguousarray(x[b].T)
        maps.append({
            "xf": xT,
            "xb": xT.astype(ml_dtypes.bfloat16),
            "wih": WihT,
            "whh": WhhT,
            "bias": biasc,
            "lwt": lwt,
            "lb": lb,
            "ident": np.eye(P, dtype=np.float32).astype(ml_dtypes.bfloat16),
        })
    return maps


def kernel(sequence_output, W_ih, W_hh, b_ih, b_hh, lin_w, lin_b):
    nc = _get_nc()
    maps = _in_maps(sequence_output, W_ih, W_hh, b_ih, b_hh, lin_w, lin_b)
    res = bass_utils.run_bass_kernel_spmd(nc, maps, core_ids=list(range(N_CORES)))
    return np.stack([res.results[b]["out"] for b in range(B)], axis=0)


def run_traced(inputs, **kw):
    """For test.py: run with NTFF tracing, returns BassKernelResults."""
    nc = _get_nc()
    maps = _in_maps(**inputs)
    return bass_utils.run_bass_kernel_spmd(
        nc, maps, core_ids=list(range(N_CORES)), trace=True, **kw
    )



# revision 8
# speedup vs baseline: 8.2928x; 8.2928x over previous
"""Trainium2 Bass kernel v2: windowed-LSTM local attention + linear head.

Data-parallel over batch: 8 sequences -> 8 NeuronCores.  Feature-on-
partitions layout: every big tile is [128 part, 512 pos].

Software pipeline over the 3 windows (sequential recurrences, overlapped
weight DMA):

  wih(0) dma -> proj(0) -> [whh(0) dma] rec(0) { proj(1), wih(2+)/whh(1) dma }
             -> rec(1) { proj(2), whh(2) dma } -> rec(2) -> softmax+head

  - wih pool bufs=1 (36KB/part): wih(k+1) DMA waits only on proj(k) MMs
  - whh pool bufs=2 (72KB/part): whh(k+1) lands while rec(k) still runs
  - Pt (gate input projections) double-buffered per tag (48KB/part)
  - identity matmul folds P+bias into the gate PSUM accumulation
  - cell state c and the whole elementwise chain in bf16 (2x DVE rate)
  - proj bias-add on ACT (Identity+bias) to keep DVE under PE
  - logits written [9, 512] contiguous; host transposes

rep>1 wraps the whole body in For_i for slope-based device timing.
"""

import math
import numpy as np
import ml_dtypes

import concourse.bacc as bacc
import concourse.bass as bass
import concourse.tile as tile
from concourse import mybir
from concourse import bass_utils

B, L, D = 8, 512, 768
NL = 9
WINDOWS = (3, 5, 7)
NW = len(WINDOWS)
G4 = 4 * D
P = 128
ND = D // P          # 6
NM = G4 // P         # 24
N_CORES = 8

F32 = mybir.dt.float32
BF16 = mybir.dt.bfloat16
FP8 = mybir.dt.float8e4
AF = mybir.ActivationFunctionType


def _emit(tc, io, rep=1, ident_mm=True, fp8_proj=False, rec_dr=True):
    nc = tc.nc
    from contextlib import ExitStack

    with ExitStack() as ctx:
        ctx.enter_context(nc.allow_low_precision(
            reason="bf16 cell state / softmax within 2e-2 rel tolerance"))
        const = ctx.enter_context(tc.tile_pool(name="const", bufs=1))
        wih_p = ctx.enter_context(tc.tile_pool(name="wih_p", bufs=1))
        whh_p = ctx.enter_context(tc.tile_pool(name="whh_p", bufs=1))
        ppool = ctx.enter_context(tc.tile_pool(name="ppool", bufs=2))
        state = ctx.enter_context(tc.tile_pool(name="state", bufs=1))
        post = ctx.enter_context(tc.tile_pool(name="post", bufs=8))
        tmp = ctx.enter_context(tc.tile_pool(name="tmp", bufs=6))
        attn = ctx.enter_context(tc.tile_pool(name="attn", bufs=1))
        psum = ctx.enter_context(tc.tile_pool(name="psum", bufs=8, space="PSUM"))

        # ---- constants resident in SBUF (outside the rep loop) ----
        xb = []   # x.T bf16: proj rhs, attention dot, head residual
        for dc in range(ND):
            t_b = const.tile([P, L], BF16, tag=f"xb{dc}", name=f"xb{dc}")
            nc.sync.dma_start(t_b, io["xb"].ap()[dc * P:(dc + 1) * P, :])
            xb.append(t_b)

        x8 = const.tile([P, ND, L], FP8, tag="x8")
        nc.sync.dma_start(x8, io["x8"].ap().rearrange("p (n l) -> p n l", n=ND))

        # bias pre-laid-out on host as [P, NW*NM]: col k*NM+m = bias[k, m*128+p]
        bias_sb = const.tile([P, NW * NM], F32, tag="bias")
        nc.sync.dma_start(bias_sb, io["bias"].ap())

        lw = []
        for dc in range(ND):
            t = const.tile([P, NL], BF16, tag=f"lw{dc}", name=f"lw{dc}")
            nc.sync.dma_start(t, io["lwt"].ap()[dc * P:(dc + 1) * P, :])
            lw.append(t)
        lb_sb = const.tile([NL, 1], F32, tag="lb")
        nc.sync.dma_start(lb_sb, io["lb"].ap().rearrange("(c o) -> c o", o=1))

        ident_sb = const.tile([P, P], BF16, tag="ident")
        nc.sync.dma_start(ident_sb, io["ident"].ap())

        ones_mat = const.tile([P, P], BF16, tag="ones_mat")
        nc.vector.memset(ones_mat, 1.0)

        # persistent state tiles (written fully each rep iteration)
        c = [state.tile([P, L], BF16, tag=f"c{dc}", name=f"c{dc}")
             for dc in range(ND)]
        h8 = state.tile([P, ND, L], FP8, tag="h8")
        hks = [[state.tile([P, L], BF16, tag=f"h{k}_{dc}", name=f"h{k}_{dc}")
                for dc in range(ND)] for k in range(NW)]
        ak_t = [state.tile([P, L], BF16, tag=f"ak{k}", name=f"akt{k}")
                for k in range(NW)]
        logits = const.tile([NL, L], F32, tag="logits")

        inv_sqrt_d = 1.0 / math.sqrt(D)

        def load_wih(k):
            t = wih_p.tile([P, ND, G4], FP8, tag="A8", name=f"A8_{k}")
            nc.sync.dma_start(
                t, io["wih"].ap()[k].rearrange("p (n g) -> p n g", n=ND))
            return t

        def load_whh(k):
            t = whh_p.tile([P, ND, G4], FP8, tag="B8", name=f"B8_{k}", bufs=2)
            nc.sync.dma_start(
                t, io["whh"].ap()[k].rearrange("p (n g) -> p n g", n=ND))
            return t

        def proj(k, wih):
            """Pt[k] = bias + Wih(k) @ x  (24 tiles, double-buffered tags)."""
            Ptk = []
            for m in range(NM):
                ps = psum.tile([P, L], F32, tag="g", name=f"pj{k}_{m}", bufs=8)
                if fp8_proj:
                    for j in range(0, ND, 2):
                        nc.tensor.matmul(
                            ps,
                            lhsT=wih[:, j:j + 2, m * P:(m + 1) * P],
                            rhs=x8[:, j:j + 2, :],
                            start=(j == 0),
                            stop=(j == ND - 2),
                            perf_mode=mybir.MatmulPerfMode.DoubleRow,
                        )
                else:
                    for j in range(ND):
                        nc.tensor.matmul(
                            ps,
                            lhsT=wih[:, j, m * P:(m + 1) * P],
                            rhs=x8[:, j, :],
                            start=(j == 0),
                            stop=(j == ND - 1),
                        )
                pt = ppool.tile([P, L], BF16, tag=f"P{m}", name=f"P{k}_{m}",
                                bufs=2)
                nc.scalar.activation(
                    pt, ps, AF.Identity,
                    bias=bias_sb[:, k * NM + m:k * NM + m + 1])
                Ptk.append(pt)
            return Ptk

        def rec_step(k, w, t, whh, h, Ptk):
            hw_ = w // 2
            off = t - hw_
            s = max(0, -off)
            e = min(L, L - off)
            n = e - s
            last = (t == w - 1)

            def h_out(dc):
                # intermediate steps feed the fp8 DoubleRow matmuls; the
                # final step lands in bf16 locals for attention/head
                return h[dc][:, s:e] if last else h8[:, dc, s:e]

            if t == 0:
                for dc in range(ND):
                    i_t = post.tile([P, L], BF16, tag="post", name="i0", bufs=8)
                    nc.scalar.activation(
                        i_t[:, :n], Ptk[0 + dc][:, s + off:e + off], AF.Sigmoid)
                    g_t = post.tile([P, L], BF16, tag="post", name="g0", bufs=8)
                    nc.scalar.activation(
                        g_t[:, :n], Ptk[12 + dc][:, s + off:e + off], AF.Tanh)
                    o_t = post.tile([P, L], BF16, tag="post", name="o0", bufs=8)
                    nc.scalar.activation(
                        o_t[:, :n], Ptk[18 + dc][:, s + off:e + off], AF.Sigmoid)
                    if s > 0:
                        nc.vector.memset(c[dc][:, 0:s], 0.0)
                        nc.vector.memset(h8[:, dc, 0:s], 0.0)
                    nc.vector.tensor_mul(c[dc][:, s:e], i_t[:, :n], g_t[:, :n])
                    tc_t = post.tile([P, L], BF16, tag="post", name="tc0", bufs=8)
                    nc.scalar.activation(tc_t[:, :n], c[dc][:, s:e], AF.Tanh)
                    nc.vector.tensor_mul(h8[:, dc, s:e], o_t[:, :n], tc_t[:, :n])
                return

            for dc in range(ND):
                gp = []
                for base in (0, 6, 12, 18):
                    m = base + dc
                    ps = psum.tile([P, L], F32, tag="g", name=f"s{t}_{m}", bufs=8)
                    nc.tensor.matmul(
                        ps[:, s:e],
                        lhsT=ident_sb[:],
                        rhs=Ptk[m][:, s + off:e + off],
                        start=True,
                        stop=False,
                    )
                    if rec_dr:
                        for j in range(0, ND, 2):
                            nc.tensor.matmul(
                                ps[:, s:e],
                                lhsT=whh[:, j:j + 2, m * P:(m + 1) * P],
                                rhs=h8[:, j:j + 2, s:e],
                                start=False,
                                stop=(j == ND - 2),
                                perf_mode=mybir.MatmulPerfMode.DoubleRow,
                            )
                    else:
                        for j in range(ND):
                            nc.tensor.matmul(
                                ps[:, s:e],
                                lhsT=whh[:, j, m * P:(m + 1) * P],
                                rhs=h8[:, j, s:e],
                                start=False,
                                stop=(j == ND - 1),
                            )
                    gp.append(ps)

                acts = []
                for gi, fn in enumerate(
                    (AF.Sigmoid, AF.Sigmoid, AF.Tanh, AF.Sigmoid)
                ):
                    a = post.tile([P, L], BF16, tag="post", name=f"a{gi}", bufs=8)
                    nc.scalar.activation(a[:, :n], gp[gi][:, s:e], fn)
                    acts.append(a)
                i_t, f_t, g_t, o_t = acts

                t1 = tmp.tile([P, L], BF16, tag="tmp", name="t1", bufs=6)
                nc.vector.tensor_mul(t1[:, :n], i_t[:, :n], g_t[:, :n])
                t2 = tmp.tile([P, L], BF16, tag="tmp", name="t2", bufs=6)
                nc.vector.tensor_mul(t2[:, :n], f_t[:, :n], c[dc][:, s:e])
                nc.vector.tensor_add(c[dc][:, s:e], t1[:, :n], t2[:, :n])
                tc_t = post.tile([P, L], BF16, tag="post", name="tct", bufs=8)
                nc.scalar.activation(tc_t[:, :n], c[dc][:, s:e], AF.Tanh)
                nc.vector.tensor_mul(h_out(dc), o_t[:, :n], tc_t[:, :n])
                if last and e < L:
                    nc.vector.tensor_copy(h[dc][:, e:L], h8[:, dc, e:L])

        def attn_dot(k, h):
            # broadcast dot: ones[P,P].T @ td accumulates the full x.h dot
            # into EVERY partition, so the 3-way softmax runs as [P, L] ops
            psd = psum.tile([P, L], F32, tag="g", name=f"dot{k}", bufs=8)
            for dc in range(ND):
                td = tmp.tile([P, L], BF16, tag="tmp", name="td", bufs=6)
                nc.vector.tensor_mul(td, xb[dc][:], h[dc][:])
                nc.tensor.matmul(
                    psd,
                    lhsT=ones_mat[:],
                    rhs=td[:],
                    start=(dc == 0),
                    stop=(dc == ND - 1),
                )
            nc.scalar.activation(ak_t[k], psd, AF.Copy, scale=inv_sqrt_d)

        def body():
            wih = load_wih(0)
            Ptk = [None] * NW
            Ptk[0] = proj(0, wih)
            whh_cur = load_whh(0)

            for k, w in enumerate(WINDOWS):
                h = hks[k]
                for t in range(w):
                    rec_step(k, w, t, whh_cur, h, Ptk[k])
                    if t == 1 and k + 1 < NW:
                        # overlap: next window's input proj + weight loads
                        wih2 = load_wih(k + 1)
                        Ptk[k + 1] = proj(k + 1, wih2)
                        whh_nxt = load_whh(k + 1)
                attn_dot(k, h)
                if k + 1 < NW:
                    whh_cur = whh_nxt

            # ===== softmax over 3 window outputs (broadcast [P, L] form) =====
            mx1 = tmp.tile([P, L], BF16, tag="tmp", name="mx1", bufs=6)
            nc.vector.tensor_max(mx1, ak_t[0][:], ak_t[1][:])
            mx2 = tmp.tile([P, L], BF16, tag="tmp", name="mx2", bufs=6)
            nc.vector.tensor_max(mx2, mx1[:], ak_t[2][:])
            e_sb = []
            for k in range(NW):
                d_k = tmp.tile([P, L], BF16, tag="tmp", name=f"dk{k}", bufs=6)
                nc.vector.tensor_sub(d_k, ak_t[k][:], mx2[:])
                ek = attn.tile([P, L], BF16, tag=f"ek{k}", name=f"ek{k}")
                nc.scalar.activation(ek, d_k, AF.Exp)
                e_sb.append(ek)
            s1 = tmp.tile([P, L], BF16, tag="tmp", name="s1", bufs=6)
            nc.vector.tensor_add(s1, e_sb[0][:], e_sb[1][:])
            s2 = tmp.tile([P, L], BF16, tag="tmp", name="s2", bufs=6)
            nc.vector.tensor_add(s2, s1[:], e_sb[2][:])
            r = attn.tile([P, L], BF16, tag="rr", name="rr")
            nc.vector.reciprocal(r, s2[:])


            ps_log = psum.tile([NL, L], F32, tag="g", name="pslog", bufs=8)
            for dc in range(ND):
                lf = tmp.tile([P, L], BF16, tag="tmp", name="lf", bufs=6)
                nc.vector.tensor_mul(lf, e_sb[0][:], hks[0][dc][:])
                t3 = tmp.tile([P, L], BF16, tag="tmp", name="t3", bufs=6)
                nc.vector.tensor_mul(t3, e_sb[1][:], hks[1][dc][:])
                lf2 = tmp.tile([P, L], BF16, tag="tmp", name="lf2", bufs=6)
                nc.vector.tensor_add(lf2, lf[:], t3[:])
                t4 = tmp.tile([P, L], BF16, tag="tmp", name="t4", bufs=6)
                nc.vector.tensor_mul(t4, e_sb[2][:], hks[2][dc][:])
                lf3 = tmp.tile([P, L], BF16, tag="tmp", name="lf3", bufs=6)
                nc.vector.tensor_add(lf3, lf2[:], t4[:])
                lf4 = tmp.tile([P, L], BF16, tag="tmp", name="lf4", bufs=6)
                nc.vector.tensor_mul(lf4, lf3[:], r[:])
                feat = tmp.tile([P, L], BF16, tag="tmp", name="feat", bufs=6)
                nc.vector.tensor_add(feat, lf4[:], xb[dc][:])
                nc.tensor.matmul(ps_log, lhsT=lw[dc][:], rhs=feat[:],
                                 start=(dc == 0), stop=(dc == ND - 1))
            nc.scalar.activation(logits, ps_log, AF.Identity, bias=lb_sb[:, 0:1])
            nc.sync.dma_start(io["out"].ap(), logits[:])

        if rep == 1:
            body()
        else:
            with tc.For_i(0, rep, 1) as _i:
                body()


_NC_CACHE = {}


def _get_nc(rep=1, ident_mm=True, fp8_proj=False, rec_dr=True):
    key = (rep, ident_mm, fp8_proj, rec_dr)
    if key not in _NC_CACHE:
        nc = bacc.Bacc("TRN2", target_bir_lowering=False, debug=False)
        io = {
            "xb": nc.dram_tensor("xb", [D, L], BF16, kind="ExternalInput"),
            "x8": nc.dram_tensor("x8", [P, ND * L], FP8, kind="ExternalInput"),
            "wih": nc.dram_tensor("wih", [NW, P, ND * G4], FP8, kind="ExternalInput"),
            "whh": nc.dram_tensor("whh", [NW, P, ND * G4], FP8, kind="ExternalInput"),
            "bias": nc.dram_tensor("bias", [P, NW * NM], F32, kind="ExternalInput"),
            "lwt": nc.dram_tensor("lwt", [D, NL], BF16, kind="ExternalInput"),
            "lb": nc.dram_tensor("lb", [NL], F32, kind="ExternalInput"),
            "ident": nc.dram_tensor("ident", [P, P], BF16, kind="ExternalInput"),
            "out": nc.dram_tensor("out", [NL, L], F32, kind="ExternalOutput"),
        }
        with tile.TileContext(nc) as tc:
            _emit(tc, io, rep=rep, ident_mm=ident_mm, fp8_proj=fp8_proj, rec_dr=rec_dr)
        nc.compile()
        _NC_CACHE[key] = nc
    return _NC_CACHE[key]


def _in_maps(sequence_output, W_ih, W_hh, b_ih, b_hh, lin_w, lin_b):
    x = np.asarray(sequence_output, np.float32)
    wih_f = np.transpose(np.asarray(W_ih, np.float32), (0, 2, 1))  # [NW, D, G4]
    Wih8 = np.ascontiguousarray(
        wih_f.reshape(NW, ND, P, G4).transpose(0, 2, 1, 3).reshape(NW, P, ND * G4)
    ).astype(ml_dtypes.float8_e4m3fn)
    whh_f = np.transpose(np.asarray(W_hh, np.float32), (0, 2, 1))  # [NW, D, G4]
    Whh8 = np.ascontiguousarray(
        whh_f.reshape(NW, ND, P, G4).transpose(0, 2, 1, 3).reshape(NW, P, ND * G4)
    ).astype(ml_dtypes.float8_e4m3fn)
    biasc = np.asarray(b_ih, np.float32) + np.asarray(b_hh, np.float32)
    bias_pm = np.ascontiguousarray(
        biasc.reshape(NW, NM, P).transpose(2, 0, 1).reshape(P, NW * NM))
    lwt = np.ascontiguousarray(
        np.asarray(lin_w, np.float32).T).astype(ml_dtypes.bfloat16)
    lb = np.asarray(lin_b, np.float32)
    ident = np.eye(P, dtype=np.float32).astype(ml_dtypes.bfloat16)
    maps = []
    for b in range(B):
        xT = np.ascontiguousarray(x[b].T)
        x8 = np.ascontiguousarray(
            xT.reshape(ND, P, L).transpose(1, 0, 2).reshape(P, ND * L)
        ).astype(ml_dtypes.float8_e4m3fn)
        maps.append({
            "xb": xT.astype(ml_dtypes.bfloat16),
            "x8": x8,
            "wih": Wih8,
            "whh": Whh8,
            "bias": bias_pm,
            "lwt": lwt,
            "lb": lb,
            "ident": ident,
        })
    return maps


def kernel(sequence_output, W_ih, W_hh, b_ih, b_hh, lin_w, lin_b):
    nc = _get_nc()
    maps = _in_maps(sequence_output, W_ih, W_hh, b_ih, b_hh, lin_w, lin_b)
    res = bass_utils.run_bass_kernel_spmd(nc, maps, core_ids=list(range(N_CORES)))
    return np.stack(
        [np.ascontiguousarray(res.results[b]["out"].T) for b in range(B)], axis=0)


# revision 10
# speedup vs baseline: 11.3454x; 1.3681x over previous
"""Trainium2 Bass kernel v2: windowed-LSTM local attention + linear head.

Data-parallel over batch: 8 sequences -> 8 NeuronCores.  Feature-on-
partitions layout: every big tile is [128 part, 512 pos].

Software pipeline over the 3 windows (sequential recurrences, overlapped
weight DMA):

  wih(0) dma -> proj(0) -> [whh(0) dma] rec(0) { proj(1), wih(2+)/whh(1) dma }
             -> rec(1) { proj(2), whh(2) dma } -> rec(2) -> softmax+head

  - wih pool bufs=1 (36KB/part): wih(k+1) DMA waits only on proj(k) MMs
  - whh pool bufs=2 (72KB/part): whh(k+1) lands while rec(k) still runs
  - Pt (gate input projections) double-buffered per tag (48KB/part)
  - identity matmul folds P+bias into the gate PSUM accumulation
  - cell state c and the whole elementwise chain in bf16 (2x DVE rate)
  - proj bias-add on ACT (Identity+bias) to keep DVE under PE
  - logits written [9, 512] contiguous; host transposes

rep>1 wraps the whole body in For_i for slope-based device timing.
"""

import math
import numpy as np
import ml_dtypes

import concourse.bacc as bacc
import concourse.bass as bass
import concourse.tile as tile
from concourse import mybir
from concourse import bass_utils

B, L, D = 8, 512, 768
NL = 9
WINDOWS = (3, 5, 7)
NW = len(WINDOWS)
G4 = 4 * D
P = 128
ND = D // P          # 6
NM = G4 // P         # 24
N_CORES = 8

F32 = mybir.dt.float32
BF16 = mybir.dt.bfloat16
FP8 = mybir.dt.float8e4
AF = mybir.ActivationFunctionType


def _emit(tc, io, rep=1, ident_mm=True, fp8_proj=False, rec_dr=True):
    nc = tc.nc
    from contextlib import ExitStack

    with ExitStack() as ctx:
        ctx.enter_context(nc.allow_low_precision(
            reason="bf16 cell state / softmax within 2e-2 rel tolerance"))
        const = ctx.enter_context(tc.tile_pool(name="const", bufs=1))
        wih_p = ctx.enter_context(tc.tile_pool(name="wih_p", bufs=1))
        whh_p = ctx.enter_context(tc.tile_pool(name="whh_p", bufs=1))
        ppool = ctx.enter_context(tc.tile_pool(name="ppool", bufs=2))
        state = ctx.enter_context(tc.tile_pool(name="state", bufs=1))
        post = ctx.enter_context(tc.tile_pool(name="post", bufs=8))
        tmp = ctx.enter_context(tc.tile_pool(name="tmp", bufs=6))
        attn = ctx.enter_context(tc.tile_pool(name="attn", bufs=1))
        psum = ctx.enter_context(tc.tile_pool(name="psum", bufs=8, space="PSUM"))

        # ---- constants resident in SBUF (outside the rep loop) ----
        xb = []   # x.T bf16: proj rhs, attention dot, head residual
        for dc in range(ND):
            t_b = const.tile([P, L], BF16, tag=f"xb{dc}", name=f"xb{dc}")
            nc.sync.dma_start(t_b, io["xb"].ap()[dc * P:(dc + 1) * P, :])
            xb.append(t_b)

        x8 = const.tile([P, ND, L], FP8, tag="x8")
        nc.sync.dma_start(x8, io["x8"].ap().rearrange("p (n l) -> p n l", n=ND))

        # bias pre-laid-out on host as [P, NW*NM]: col k*NM+m = bias[k, m*128+p]
        bias_sb = const.tile([P, NW * NM], F32, tag="bias")
        nc.sync.dma_start(bias_sb, io["bias"].ap())

        lw = []
        for dc in range(ND):
            t = const.tile([P, NL], BF16, tag=f"lw{dc}", name=f"lw{dc}")
            nc.sync.dma_start(t, io["lwt"].ap()[dc * P:(dc + 1) * P, :])
            lw.append(t)
        lb_sb = const.tile([NL, 1], F32, tag="lb")
        nc.sync.dma_start(lb_sb, io["lb"].ap().rearrange("(c o) -> c o", o=1))

        ident_sb = const.tile([P, P], BF16, tag="ident")
        nc.sync.dma_start(ident_sb, io["ident"].ap())

        ones_mat = const.tile([P, P], BF16, tag="ones_mat")
        nc.vector.memset(ones_mat, 1.0)

        # persistent state tiles (written fully each rep iteration)
        c = [state.tile([P, L], BF16, tag=f"c{dc}", name=f"c{dc}")
             for dc in range(ND)]
        h8 = state.tile([P, ND, L], FP8, tag="h8")
        hks = [[state.tile([P, L], BF16, tag=f"h{k}_{dc}", name=f"h{k}_{dc}")
                for dc in range(ND)] for k in range(NW)]
        ak_t = [state.tile([P, L], BF16, tag=f"ak{k}", name=f"akt{k}")
                for k in range(NW)]
        logits = const.tile([NL, L], F32, tag="logits")

        inv_sqrt_d = 1.0 / math.sqrt(D)

        def load_wih(k):
            t = wih_p.tile([P, ND, G4], FP8, tag="A8", name=f"A8_{k}")
            nc.sync.dma_start(
                t, io["wih"].ap()[k].rearrange("p (n g) -> p n g", n=ND))
            return t

        def load_whh(k):
            t = whh_p.tile([P, ND, G4], FP8, tag="B8", name=f"B8_{k}", bufs=2)
            nc.sync.dma_start(
                t, io["whh"].ap()[k].rearrange("p (n g) -> p n g", n=ND))
            return t

        def proj(k, wih):
            """Pt[k] = bias + Wih(k) @ x  (24 tiles, double-buffered tags)."""
            Ptk = []
            for m in range(NM):
                ps = psum.tile([P, L], F32, tag="g", name=f"pj{k}_{m}", bufs=8)
                if fp8_proj:
                    for j in range(0, ND, 2):
                        nc.tensor.matmul(
                            ps,
                            lhsT=wih[:, j:j + 2, m * P:(m + 1) * P],
                            rhs=x8[:, j:j + 2, :],
                            start=(j == 0),
                            stop=(j == ND - 2),
                            perf_mode=mybir.MatmulPerfMode.DoubleRow,
                        )
                else:
                    for j in range(ND):
                        nc.tensor.matmul(
                            ps,
                            lhsT=wih[:, j, m * P:(m + 1) * P],
                            rhs=x8[:, j, :],
                            start=(j == 0),
                            stop=(j == ND - 1),
                        )
                pt = ppool.tile([P, L], BF16, tag=f"P{m}", name=f"P{k}_{m}",
                                bufs=2)
                nc.vector.tensor_scalar_add(
                    pt, ps, bias_sb[:, k * NM + m:k * NM + m + 1])
                Ptk.append(pt)
            return Ptk

        def rec_step(k, w, t, whh, h, Ptk):
            hw_ = w // 2
            off = t - hw_
            s = max(0, -off)
            e = min(L, L - off)
            n = e - s
            last = (t == w - 1)

            def h_out(dc):
                # intermediate steps feed the fp8 DoubleRow matmuls; the
                # final step lands in bf16 locals for attention/head
                return h[dc][:, s:e] if last else h8[:, dc, s:e]

            if t == 0:
                for dc in range(ND):
                    i_t = post.tile([P, L], BF16, tag="post", name="i0", bufs=8)
                    nc.scalar.activation(
                        i_t[:, :n], Ptk[0 + dc][:, s + off:e + off], AF.Sigmoid)
                    g_t = post.tile([P, L], BF16, tag="post", name="g0", bufs=8)
                    nc.scalar.activation(
                        g_t[:, :n], Ptk[12 + dc][:, s + off:e + off], AF.Tanh)
                    o_t = post.tile([P, L], BF16, tag="post", name="o0", bufs=8)
                    nc.scalar.activation(
                        o_t[:, :n], Ptk[18 + dc][:, s + off:e + off], AF.Sigmoid)
                    if s > 0:
                        nc.vector.memset(c[dc][:, 0:s], 0.0)
                        nc.vector.memset(h8[:, dc, 0:s], 0.0)
                    nc.vector.tensor_mul(c[dc][:, s:e], i_t[:, :n], g_t[:, :n])
                    tc_t = post.tile([P, L], BF16, tag="post", name="tc0", bufs=8)
                    nc.scalar.activation(tc_t[:, :n], c[dc][:, s:e], AF.Tanh)
                    nc.vector.tensor_mul(h8[:, dc, s:e], o_t[:, :n], tc_t[:, :n])
                return

            for dc in range(ND):
                gp = []
                for base in (0, 6, 12, 18):
                    m = base + dc
                    ps = psum.tile([P, L], F32, tag="g", name=f"s{t}_{m}", bufs=8)
                    nc.tensor.matmul(
                        ps[:, s:e],
                        lhsT=ident_sb[:],
                        rhs=Ptk[m][:, s + off:e + off],
                        start=True,
                        stop=False,
                    )
                    if rec_dr:
                        for j in range(0, ND, 2):
                            nc.tensor.matmul(
                                ps[:, s:e],
                                lhsT=whh[:, j:j + 2, m * P:(m + 1) * P],
                                rhs=h8[:, j:j + 2, s:e],
                                start=False,
                                stop=(j == ND - 2),
                                perf_mode=mybir.MatmulPerfMode.DoubleRow,
                            )
                    else:
                        for j in range(ND):
                            nc.tensor.matmul(
                                ps[:, s:e],
                                lhsT=whh[:, j, m * P:(m + 1) * P],
                                rhs=h8[:, j, s:e],
                                start=False,
                                stop=(j == ND - 1),
                            )
                    gp.append(ps)

                acts = []
                for gi, fn in enumerate(
                    (AF.Sigmoid, AF.Sigmoid, AF.Tanh, AF.Sigmoid)
                ):
                    a = post.tile([P, L], BF16, tag="post", name=f"a{gi}", bufs=8)
                    nc.scalar.activation(a[:, :n], gp[gi][:, s:e], fn)
                    acts.append(a)
                i_t, f_t, g_t, o_t = acts

                t1 = tmp.tile([P, L], BF16, tag="tmp", name="t1", bufs=6)
                nc.vector.tensor_mul(t1[:, :n], i_t[:, :n], g_t[:, :n])
                t2 = tmp.tile([P, L], BF16, tag="tmp", name="t2", bufs=6)
                nc.vector.tensor_mul(t2[:, :n], f_t[:, :n], c[dc][:, s:e])
                nc.vector.tensor_add(c[dc][:, s:e], t1[:, :n], t2[:, :n])
                tc_t = post.tile([P, L], BF16, tag="post", name="tct", bufs=8)
                nc.scalar.activation(tc_t[:, :n], c[dc][:, s:e], AF.Tanh)
                nc.vector.tensor_mul(h_out(dc), o_t[:, :n], tc_t[:, :n])
                if last and e < L:
                    nc.vector.tensor_copy(h[dc][:, e:L], h8[:, dc, e:L])

        def attn_dot(k, h):
            # broadcast dot: ones[P,P].T @ td accumulates the full x.h dot
            # into EVERY partition, so the 3-way softmax runs as [P, L] ops
            psd = psum.tile([P, L], F32, tag="g", name=f"dot{k}", bufs=8)
            for dc in range(ND):
                td = tmp.tile([P, L], BF16, tag="tmp", name="td", bufs=6)
                nc.vector.tensor_mul(td, xb[dc][:], h[dc][:])
                nc.tensor.matmul(
                    psd,
                    lhsT=ones_mat[:],
                    rhs=td[:],
                    start=(dc == 0),
                    stop=(dc == ND - 1),
                )
            nc.scalar.activation(ak_t[k], psd, AF.Copy, scale=inv_sqrt_d)

        def body():
            wih = load_wih(0)
            Ptk = [None] * NW
            Ptk[0] = proj(0, wih)
            whh_cur = load_whh(0)

            for k, w in enumerate(WINDOWS):
                h = hks[k]
                for t in range(w):
                    rec_step(k, w, t, whh_cur, h, Ptk[k])
                    if t == 1 and k + 1 < NW:
                        # overlap: next window's input proj + weight loads
                        wih2 = load_wih(k + 1)
                        Ptk[k + 1] = proj(k + 1, wih2)
                        whh_nxt = load_whh(k + 1)
                attn_dot(k, h)
                if k + 1 < NW:
                    whh_cur = whh_nxt

            # ===== softmax over 3 window outputs (broadcast [P, L] form) =====
            mx1 = tmp.tile([P, L], BF16, tag="tmp", name="mx1", bufs=6)
            nc.vector.tensor_max(mx1, ak_t[0][:], ak_t[1][:])
            mx2 = tmp.tile([P, L], BF16, tag="tmp", name="mx2", bufs=6)
            nc.vector.tensor_max(mx2, mx1[:], ak_t[2][:])
            e_sb = []
            for k in range(NW):
                d_k = tmp.tile([P, L], BF16, tag="tmp", name=f"dk{k}", bufs=6)
                nc.vector.tensor_sub(d_k, ak_t[k][:], mx2[:])
                ek = attn.tile([P, L], BF16, tag=f"ek{k}", name=f"ek{k}")
                nc.scalar.activation(ek, d_k, AF.Exp)
                e_sb.append(ek)
            s1 = tmp.tile([P, L], BF16, tag="tmp", name="s1", bufs=6)
            nc.vector.tensor_add(s1, e_sb[0][:], e_sb[1][:])
            s2 = tmp.tile([P, L], BF16, tag="tmp", name="s2", bufs=6)
            nc.vector.tensor_add(s2, s1[:], e_sb[2][:])
            r = attn.tile([P, L], BF16, tag="rr", name="rr")
            nc.vector.reciprocal(r, s2[:])


            ps_log = psum.tile([NL, L], F32, tag="g", name="pslog", bufs=8)
            for dc in range(ND):
                lf = tmp.tile([P, L], BF16, tag="tmp", name="lf", bufs=6)
                nc.vector.tensor_mul(lf, e_sb[0][:], hks[0][dc][:])
                t3 = tmp.tile([P, L], BF16, tag="tmp", name="t3", bufs=6)
                nc.vector.tensor_mul(t3, e_sb[1][:], hks[1][dc][:])
                lf2 = tmp.tile([P, L], BF16, tag="tmp", name="lf2", bufs=6)
                nc.vector.tensor_add(lf2, lf[:], t3[:])
                t4 = tmp.tile([P, L], BF16, tag="tmp", name="t4", bufs=6)
                nc.vector.tensor_mul(t4, e_sb[2][:], hks[2][dc][:])
                lf3 = tmp.tile([P, L], BF16, tag="tmp", name="lf3", bufs=6)
                nc.vector.tensor_add(lf3, lf2[:], t4[:])
                lf4 = tmp.tile([P, L], BF16, tag="tmp", name="lf4", bufs=6)
                nc.vector.tensor_mul(lf4, lf3[:], r[:])
                feat = tmp.tile([P, L], BF16, tag="tmp", name="feat", bufs=6)
                nc.vector.tensor_add(feat, lf4[:], xb[dc][:])
                nc.tensor.matmul(ps_log, lhsT=lw[dc][:], rhs=feat[:],
                                 start=(dc == 0), stop=(dc == ND - 1))
            nc.scalar.activation(logits, ps_log, AF.Identity, bias=lb_sb[:, 0:1])
            nc.sync.dma_start(io["out"].ap(), logits[:])

        if rep == 1:
            body()
        else:
            with tc.For_i(0, rep, 1) as _i:
                body()


_NC_CACHE = {}


def _get_nc(rep=1, ident_mm=True, fp8_proj=False, rec_dr=True):
    key = (rep, ident_mm, fp8_proj, rec_dr)
    if key not in _NC_CACHE:
        nc = bacc.Bacc("TRN2", target_bir_lowering=False, debug=False)
        io = {
            "xb": nc.dram_tensor("xb", [D, L], BF16, kind="ExternalInput"),
            "x8": nc.dram_tensor("x8", [P, ND * L], FP8, kind="ExternalInput"),
            "wih": nc.dram_tensor("wih", [NW, P, ND * G4], FP8, kind="ExternalInput"),
            "whh": nc.dram_tensor("whh", [NW, P, ND * G4], FP8, kind="ExternalInput"),
            "bias": nc.dram_tensor("bias", [P, NW * NM], F32, kind="ExternalInput"),
            "lwt": nc.dram_tensor("lwt", [D, NL], BF16, kind="ExternalInput"),
            "lb": nc.dram_tensor("lb", [NL], F32, kind="ExternalInput"),
            "ident": nc.dram_tensor("ident", [P, P], BF16, kind="ExternalInput"),
            "out": nc.dram_tensor("out", [NL, L], F32, kind="ExternalOutput"),
        }
        with tile.TileContext(nc) as tc:
            _emit(tc, io, rep=rep, ident_mm=ident_mm, fp8_proj=fp8_proj, rec_dr=rec_dr)
        nc.compile()
        _NC_CACHE[key] = nc
    return _NC_CACHE[key]


def _in_maps(sequence_output, W_ih, W_hh, b_ih, b_hh, lin_w, lin_b):
    x = np.asarray(sequence_output, np.float32)
    wih_f = np.transpose(np.asarray(W_ih, np.float32), (0, 2, 1))  # [NW, D, G4]
    Wih8 = np.ascontiguousarray(
        wih_f.reshape(NW, ND, P, G4).transpose(0, 2, 1, 3).reshape(NW, P, ND * G4)
    ).astype(ml_dtypes.float8_e4m3fn)
    whh_f = np.transpose(np.asarray(W_hh, np.float32), (0, 2, 1))  # [NW, D, G4]
    Whh8 = np.ascontiguousarray(
        whh_f.reshape(NW, ND, P, G4).transpose(0, 2, 1, 3).reshape(NW, P, ND * G4)
    ).astype(ml_dtypes.float8_e4m3fn)
    biasc = np.asarray(b_ih, np.float32) + np.asarray(b_hh, np.float32)
    bias_pm = np.ascontiguousarray(
        biasc.reshape(NW, NM, P).transpose(2, 0, 1).reshape(P, NW * NM))
    lwt = np.ascontiguousarray(
        np.asarray(lin_w, np.float32).T).astype(ml_dtypes.bfloat16)
    lb = np.asarray(lin_b, np.float32)
    ident = np.eye(P, dtype=np.float32).astype(ml_dtypes.bfloat16)
    maps = []
    for b in range(B):
        xT = np.ascontiguousarray(x[b].T)
        x8 = np.ascontiguousarray(
            xT.reshape(ND, P, L).transpose(1, 0, 2).reshape(P, ND * L)
        ).astype(ml_dtypes.float8_e4m3fn)
        maps.append({
            "xb": xT.astype(ml_dtypes.bfloat16),
            "x8": x8,
            "wih": Wih8,
            "whh": Whh8,
            "bias": bias_pm,
            "lwt": lwt,
            "lb": lb,
            "ident": ident,
        })
    return maps


def kernel(sequence_output, W_ih, W_hh, b_ih, b_hh, lin_w, lin_b):
    nc = _get_nc()
    maps = _in_maps(sequence_output, W_ih, W_hh, b_ih, b_hh, lin_w, lin_b)
    res = bass_utils.run_bass_kernel_spmd(nc, maps, core_ids=list(range(N_CORES)))
    return np.stack(
        [np.ascontiguousarray(res.results[b]["out"].T) for b in range(B)], axis=0)
